# revision 20
# baseline (speedup 1.0000x reference)
"""AERIALAgent distributed Trainium2 kernel (8 NeuronCores).

Strategy (row/data parallel over the agent axis):
  - Each core owns 1024 of the 8192 agents: obs MLP, attention rows, GRU
    and outputs for its slice.
  - The belief projection bp = beliefs @ Wb + bb (8192x64, 134 MFLOP ~ 0.7%
    of total work) is computed once on the host in bf16 and REPLICATED to
    all cores in the layouts the attention needs (bpT, bp_aug tiles). In
    this environment a 1MB AllGather costs 60-90us (ncfw rendezvous barrier
    + RDH), dwarfing the projection itself, so replication beats the
    collective by a wide margin and removes all cross-core dependencies.
  - The 8192x8192 attention is flash-style per core with TRANSPOSED score
    tiles ST[j, a] so the softmax-weighted accumulation needs no on-device
    transposes. QK pairs run concurrently on the PE via row tiling
    (tile_position (0,0)/(64,0), K=64 each).
  - Softmax runs without max subtraction (logits in [-3, 6]); the diagonal
    mask is applied algebraically by subtracting exp(|bp_a|^2/8) terms from
    the context numerator and denominator after accumulation.
  - GRU runs fully in transposed [hidden, agent] layout with composite
    weights (Wg @ Wi*) folded on the host; sigmoid is computed via tanh so
    one ACT table set (exp/tanh/relu/copy) serves the whole kernel.
  - Attention matmuls and the exp output run in fp8-e4m3 (f32 PSUM
    accumulation); the PV pair uses the fp8 DoubleRow perf mode. GRU/MLP
    matmuls are bf16. Outputs return transposed; the host transposes back.
"""

import numpy as np
import ml_dtypes

import concourse.bass as bass
import concourse.bacc as bacc
import concourse.mybir as mybir
import concourse.tile as tile
from concourse.bass_utils import run_bass_kernel_spmd

BF16_NP = ml_dtypes.bfloat16
F8_NP = ml_dtypes.float8_e4m3
F32 = mybir.dt.float32
BF16 = mybir.dt.bfloat16
F8 = mybir.dt.float8e4
AF = mybir.ActivationFunctionType
ALU = mybir.AluOpType

CORES = 8
A = 8192           # total agents
AL = A // CORES    # 1024 agents per core
OBS = 520
OBSP = 640         # obs dim padded to 5 * 128
KC = OBSP // 128   # 5 contraction chunks for the obs matmul
E = 64             # embed
H = 128            # hidden
NACT = 6
JT = A // 128      # 64 key tiles
NSB = AL // 512    # 2 superblocks of 512 query agents
JSTR = 80          # bp_aug per-tile column stride (64 data + ones + pad, 16-aligned for DoubleRow)
NBPT = 4           # bpT2 / bp_aug split into 4 tiles for DMA/compute overlap

# wpack column offsets (bf16 [128, WCOLS])
W1_O = 0              # 5 chunks of 64
W2_O = 320
WGR_O = 384
WGZ_O = 512
WGN_O = 640
WHR_O = 768
WHZ_O = 896
WHN_O = 1024          # 0.5 * Whn
BELB_O = 1152         # own beliefs.T bf16 [128, 1024]
WCOLS = BELB_O + AL

# brow column offsets (bf16 [1, BCOLS])
B1_O = 0
B2_O = 64
BR_O = 128            # bhr + bg @ Wir
BZ_O = 256            # bhz + bg @ Wiz
BHN_O = 384           # 0.5 * bhn
BGN_O = 512           # bg @ Win
BOUT_O = 640
BCOLS = 648


def _build_nc():
    nc = bacc.Bacc(
        "TRN2",
        target_bir_lowering=False,
        debug=False,
        num_devices=CORES,
    )

    obsT_d = nc.dram_tensor("obsT", [128, KC * AL], BF16, kind="ExternalInput")
    belT_d = nc.dram_tensor("belT", [H, AL], F32, kind="ExternalInput")
    wpack_d = nc.dram_tensor("wpack", [128, WCOLS], BF16, kind="ExternalInput")
    wout_d = nc.dram_tensor("wout", [H, NACT], BF16, kind="ExternalInput")
    bpT2_d = nc.dram_tensor("bpT2", [128, A], F8, kind="ExternalInput")
    bpaug_d = nc.dram_tensor("bpaug", [128, JT * JSTR], F8, kind="ExternalInput")
    bpTl2_d = nc.dram_tensor("bpTl2", [128, AL], F8, kind="ExternalInput")
    expd_d = nc.dram_tensor("expd", [1, AL], F32, kind="ExternalInput")
    bpexp_d = nc.dram_tensor("bpexp", [E, AL], BF16, kind="ExternalInput")
    bcol_d = nc.dram_tensor("bcol", [128, 8], F32, kind="ExternalInput")

    out_nbT_d = nc.dram_tensor("out_nbT", [H, AL], F32, kind="ExternalOutput")
    out_lg_d = nc.dram_tensor("out_lg", [NACT, AL], F32, kind="ExternalOutput")

    JPT = JT // NBPT  # key tiles per bpT2/bpaug tile

    with tile.TileContext(nc) as tc:
        with (
            tc.tile_pool(name="const", bufs=1) as const,
            tc.tile_pool(name="work", bufs=2) as work,
            tc.tile_pool(name="expp", bufs=4) as expp,
            tc.tile_pool(name="pst", bufs=2, space=bass.MemorySpace.PSUM) as pst,
            tc.tile_pool(name="pctx", bufs=1, space=bass.MemorySpace.PSUM) as pctx,
            tc.tile_pool(name="pmisc", bufs=1, space=bass.MemorySpace.PSUM) as pmisc,
        ):
            # ---------------- inputs (attention-critical DMAs first) ----------------
            bpTl2_sb = const.tile([128, AL], F8, name="bpTl2_sb")
            nc.sync.dma_start(bpTl2_sb[:], bpTl2_d[:])
            bpT2_sb = const.tile([128, A], F8, name="bpT2_sb")
            bpaug_sb = const.tile([128, JT * JSTR], F8, name="bpaug_sb")
            for lo, hi in ((0, 1024), (1024, 4096), (4096, 8192)):
                nc.sync.dma_start(bpT2_sb[:, lo:hi], bpT2_d[:, lo:hi])
                jlo, jhi = lo // 128 * JSTR, hi // 128 * JSTR
                nc.sync.dma_start(bpaug_sb[:, jlo:jhi], bpaug_d[:, jlo:jhi])

            w_sb = const.tile([128, WCOLS], BF16, name="w_sb")
            obsT_sb = const.tile([128, KC * AL], BF16, name="obsT_sb")
            belT_sb = const.tile([H, AL], F32, name="belT_sb")
            wout_sb = const.tile([H, NACT], BF16, name="wout_sb")
            expd_sb = const.tile([1, AL], F32, name="expd_sb")

            nc.gpsimd.dma_start(w_sb[:], wpack_d[:])
            nc.gpsimd.dma_start(obsT_sb[:], obsT_d[:])
            nc.gpsimd.dma_start(belT_sb[:], belT_d[:])
            nc.gpsimd.dma_start(wout_sb[:], wout_d[:])
            nc.gpsimd.dma_start(expd_sb[:], expd_d[:])
            bpexp_sb = const.tile([E, AL], BF16, name="bpexp_sb")
            bcol_sb = const.tile([128, 8], F32, name="bcol_sb")
            nc.gpsimd.dma_start(bpexp_sb[:], bpexp_d[:])
            nc.gpsimd.dma_start(bcol_sb[:], bcol_d[:])

            ones_sb = const.tile([1, 512], BF16, name="ones_sb")
            nc.vector.memset(ones_sb[:], 1.0)

            h1T_sb = const.tile([E, AL], BF16, name="h1T_sb")
            concatT_sb = const.tile([H, AL], BF16, name="concatT_sb")
            nbT_sb = const.tile([H, AL], F32, name="nbT_sb")
            lg_sb = const.tile([NACT, AL], F32, name="lg_sb")
            den_sb = const.tile([1, AL], F32, name="den_sb")
            recip_sb = const.tile([1, AL], F32, name="recip_sb")
            recipb_sb = const.tile([1, AL], BF16, name="recipb_sb")

            ctx_tiles = [
                pctx.tile([128, 512], F32, name=f"ctx{sb}", tag="ctx")
                for sb in range(NSB)
            ]

            NG = (JT + 2) // 3  # attention groups of up to 3 key tiles

            def attn_groups(sb, g_lo, g_hi):
                """Emit attention groups [g_lo, g_hi) for superblock sb.

                QK: K=64 matmuls, pairs overlapped in PE row halves.
                PV: one fp8 DoubleRow pair + one regular matmul."""
                ctx = ctx_tiles[sb]
                cs = slice(sb * 512, (sb + 1) * 512)
                for g in range(g_lo, g_hi):
                    jts = list(range(3 * g, min(3 * g + 3, JT)))
                    st = pst.tile([128, 1536], F32, name="st", tag="st")
                    for i, jt in enumerate(jts):
                        half = i % 2  # alternate PE row groups so pairs overlap
                        nc.tensor.matmul(
                            st[:, i * 512 : (i + 1) * 512],
                            bpT2_sb[
                                half * E : (half + 1) * E,
                                jt * 128 : (jt + 1) * 128,
                            ],
                            bpTl2_sb[half * E : (half + 1) * E, cs],
                            start=True,
                            stop=True,
                            tile_position=(half * E, 0),
                        )
                    ex = expp.tile([128, 1536], F8, name="ex")
                    nc.scalar.activation(
                        ex[:, 0 : len(jts) * 512],
                        st[:, 0 : len(jts) * 512],
                        AF.Exp,
                        scale=0.125,
                    )
                    if len(jts) >= 2:
                        # PV for jts[0], jts[1] fused in one DoubleRow matmul
                        nc.tensor.matmul(
                            ctx[0 : E + 1, :],
                            bpaug_sb[
                                :, jts[0] * JSTR : jts[0] * JSTR + 160
                            ].rearrange("p (i m) -> p i m", i=2)[:, :, 0 : E + 1],
                            ex[:, 0:1024].rearrange("p (i n) -> p i n", i=2),
                            start=(jts[0] == 0),
                            stop=(jts[-1] == JT - 1 and len(jts) == 2),
                            perf_mode=mybir.MatmulPerfMode.DoubleRow,
                        )
                    for i, jt in enumerate(jts[2:], start=2):
                        nc.tensor.matmul(
                            ctx[0 : E + 1, :],
                            bpaug_sb[:, jt * JSTR : jt * JSTR + E + 1],
                            ex[:, i * 512 : (i + 1) * 512],
                            start=(jt == 0),
                            stop=(jt == JT - 1),
                        )
                    if len(jts) == 1:
                        nc.tensor.matmul(
                            ctx[0 : E + 1, :],
                            bpaug_sb[:, jts[0] * JSTR : jts[0] * JSTR + E + 1],
                            ex[:, 0:512],
                            start=(jts[0] == 0),
                            stop=(jts[0] == JT - 1),
                        )

            def denom_fix(sb):
                """Denominator (diag removed) and its reciprocal — DVE only."""
                cs = slice(sb * 512, (sb + 1) * 512)
                ctx = ctx_tiles[sb]
                nc.vector.tensor_tensor(
                    den_sb[:, cs], ctx[E : E + 1, :], expd_sb[:, cs], ALU.subtract
                )
                nc.vector.reciprocal_approx_fast(recip_sb[:, cs], den_sb[:, cs])
                nc.vector.tensor_copy(recipb_sb[:, cs], recip_sb[:, cs])

            def obs_mlp():
                # h2T = relu(W2.T relu(W1.T obsT + b1) + b2) -> concatT rows 0..63
                for ch in range(2):
                    cs = slice(ch * 512, (ch + 1) * 512)
                    p1 = pmisc.tile([128, 512], F32, name="pob1", tag="pm")
                    for kc in range(KC):
                        nc.tensor.matmul(
                            p1[0:E, :],
                            w_sb[:, W1_O + kc * E : W1_O + (kc + 1) * E],
                            obsT_sb[:, kc * AL + ch * 512 : kc * AL + (ch + 1) * 512],
                            start=(kc == 0),
                            stop=(kc == KC - 1),
                        )
                    nc.vector.tensor_scalar(
                        h1T_sb[:, cs], p1[0:E, :], bcol_sb[0:E, 3:4], 0.0,
                        ALU.add, ALU.max,
                    )
                for ch in range(2):
                    cs = slice(ch * 512, (ch + 1) * 512)
                    p2 = pmisc.tile([128, 512], F32, name="pob2", tag="pm")
                    nc.tensor.matmul(
                        p2[0:E, :],
                        w_sb[0:E, W2_O : W2_O + E],
                        h1T_sb[:, cs],
                        start=True,
                        stop=True,
                    )
                    nc.vector.tensor_scalar(
                        concatT_sb[0:E, cs], p2[0:E, :], bcol_sb[0:E, 4:5], 0.0,
                        ALU.add, ALU.max,
                    )

            def tail_fix(sb):
                """Normalize the context and write it into concatT rows 64..127."""
                cs = slice(sb * 512, (sb + 1) * 512)
                ctx = ctx_tiles[sb]
                rb = pmisc.tile([128, 512], F32, name="prb", tag="pm")
                nc.tensor.matmul(
                    rb[0:E, :], ones_sb[:, 0:E], recipb_sb[:, cs], start=True, stop=True
                )
                t_sb = work.tile([E, 512], F32, name="t_sb")
                nc.vector.tensor_tensor(t_sb[:], ctx[0:E, :], bpexp_sb[:, cs], ALU.subtract)
                nc.vector.tensor_tensor(
                    concatT_sb[E:H, cs], t_sb[:], rb[0:E, :], ALU.mult
                )

            def tail_gru(sb):
                """GRU chain producing new beliefs (transposed layout).

                The first superblock's chain overlaps attention and uses the
                single misc PSUM bank; the final one runs after attention, so
                it borrows the (now idle) st pool for pipelining."""
                cs = slice(sb * 512, (sb + 1) * 512)
                belb_rhs = w_sb[:, BELB_O + sb * 512 : BELB_O + (sb + 1) * 512]
                last = sb == NSB - 1

                def gpsum(nm):
                    if last:
                        return pst.tile([128, 512], F32, name=nm + "s", tag="st")
                    return pmisc.tile([128, 512], F32, name=nm, tag="pm")

                # u-gate first: it only needs beliefs, so it runs while the
                # context fix is still finishing
                pu = gpsum("pgu")
                nc.tensor.matmul(pu[:], w_sb[:, WHN_O : WHN_O + H], belb_rhs,
                                 start=True, stop=True)
                u2_sb = work.tile([H, 512], F32, name="u2_sb")
                nc.vector.tensor_scalar(u2_sb[:], pu[:], bcol_sb[:, 6:7], None, ALU.add)

                pr = gpsum("pgr")
                nc.tensor.matmul(pr[:], w_sb[:, WGR_O : WGR_O + H], concatT_sb[:, cs],
                                 start=True, stop=False)
                nc.tensor.matmul(pr[:], w_sb[:, WHR_O : WHR_O + H], belb_rhs,
                                 start=False, stop=True)
                tr_sb = work.tile([H, 512], F32, name="tr_sb")
                nc.scalar.activation(tr_sb[:], pr[:], AF.Tanh, bias=bcol_sb[:, 0:1], scale=0.5)

                pz = gpsum("pgz")
                nc.tensor.matmul(pz[:], w_sb[:, WGZ_O : WGZ_O + H], concatT_sb[:, cs],
                                 start=True, stop=False)
                nc.tensor.matmul(pz[:], w_sb[:, WHZ_O : WHZ_O + H], belb_rhs,
                                 start=False, stop=True)
                tz_sb = work.tile([H, 512], F32, name="tz_sb")
                nc.scalar.activation(tz_sb[:], pz[:], AF.Tanh, bias=bcol_sb[:, 1:2], scale=0.5)

                pn = gpsum("pgn")
                nc.tensor.matmul(pn[:], w_sb[:, WGN_O : WGN_O + H], concatT_sb[:, cs],
                                 start=True, stop=True)
                # v = (tr + 1) * u'   (u' = 0.5*(bel@Whn + bhn))
                v_sb = work.tile([H, 512], F32, name="v_sb")
                nc.vector.scalar_tensor_tensor(
                    v_sb[:], tr_sb[:], 1.0, u2_sb[:], ALU.add, ALU.mult
                )
                t2_sb = work.tile([H, 512], F32, name="t2_sb")
                nc.vector.tensor_tensor(t2_sb[:], v_sb[:], pn[:], ALU.add)
                n_sb = work.tile([H, 512], F32, name="n_sb")
                nc.scalar.activation(n_sb[:], t2_sb[:], AF.Tanh, bias=bcol_sb[:, 2:3])

                # new_beliefs = n + 0.5*(tz + 1)*(beliefs - n)
                d_sb = work.tile([H, 512], F32, name="d_sb")
                nc.vector.tensor_tensor(d_sb[:], belT_sb[:, cs], n_sb[:], ALU.subtract)
                w2_sb = work.tile([H, 512], F32, name="w2_sb")
                nc.vector.scalar_tensor_tensor(
                    w2_sb[:], tz_sb[:], 1.0, d_sb[:], ALU.add, ALU.mult
                )
                nc.vector.scalar_tensor_tensor(
                    nbT_sb[:, cs], w2_sb[:], 0.5, n_sb[:], ALU.mult, ALU.add
                )

            def tail_out(sb):
                cs = slice(sb * 512, (sb + 1) * 512)
                # logits, transposed: lgT = Wout.T @ nbT + bout
                nbTb_sb = work.tile([H, 512], BF16, name="nbTb_sb")
                nc.vector.tensor_copy(nbTb_sb[:], nbT_sb[:, cs])
                plg = pmisc.tile([128, 512], F32, name="plg", tag="pm")
                nc.tensor.matmul(plg[0:NACT, :], wout_sb[:], nbTb_sb[:],
                                 start=True, stop=True)
                nc.scalar.activation(
                    lg_sb[:, cs], plg[0:NACT, :], AF.Identity,
                    bias=bcol_sb[0:NACT, 5:6],
                )
                nc.sync.dma_start(out_nbT_d[:, cs], nbT_sb[:, cs])
                nc.sync.dma_start(out_lg_d[:, cs], lg_sb[:, cs])

            # Emission order: keep the PE queue dense with attention while the
            # sb0 tail's dependencies resolve, then slot tail work into the
            # middle of sb1's stream so DVE/ACT tail ops overlap attention.
            attn_groups(0, 0, NG)
            denom_fix(0)
            tail_fix(0)
            obs_mlp()
            attn_groups(1, 0, 6)
            tail_gru(0)
            attn_groups(1, 6, NG)
            denom_fix(1)
            tail_fix(1)
            tail_out(0)
            tail_gru(1)
            tail_out(1)

    nc.compile()
    return nc


_NC_CACHE = {}


def _get_nc():
    if "nc" not in _NC_CACHE:
        _NC_CACHE["nc"] = _build_nc()
    return _NC_CACHE["nc"]


def _prep_inputs(inputs):
    f32 = np.float32
    obs = np.asarray(inputs["obs"], f32)
    beliefs = np.asarray(inputs["beliefs"], f32)
    W1 = np.asarray(inputs["W1"], f32)
    b1 = np.asarray(inputs["b1"], f32)
    W2 = np.asarray(inputs["W2"], f32)
    b2 = np.asarray(inputs["b2"], f32)
    Wb = np.asarray(inputs["Wb"], f32)
    bb = np.asarray(inputs["bb"], f32)
    Wg = np.asarray(inputs["Wg"], f32)
    bg = np.asarray(inputs["bg"], f32)
    Wir = np.asarray(inputs["Wir"], f32)
    Wiz = np.asarray(inputs["Wiz"], f32)
    Win = np.asarray(inputs["Win"], f32)
    Whr = np.asarray(inputs["Whr"], f32)
    bhr = np.asarray(inputs["bhr"], f32)
    Whz = np.asarray(inputs["Whz"], f32)
    bhz = np.asarray(inputs["bhz"], f32)
    Whn = np.asarray(inputs["Whn"], f32)
    bhn = np.asarray(inputs["bhn"], f32)
    Wout = np.asarray(inputs["Wout"], f32)
    bout = np.asarray(inputs["bout"], f32)

    # composite GRU input weights (gru_in never materializes on device)
    Wgr = Wg @ Wir
    Wgz = Wg @ Wiz
    Wgn = Wg @ Win
    bgr = bg @ Wir
    bgz = bg @ Wiz
    bgn = bg @ Win

    wcommon = np.zeros((128, BELB_O), BF16_NP)
    W1p = np.zeros((OBSP, E), f32)
    W1p[:OBS] = W1
    wcommon[:, W1_O : W1_O + KC * E] = (
        W1p.reshape(KC, 128, E).transpose(1, 0, 2).reshape(128, KC * E).astype(BF16_NP)
    )
    wcommon[0:E, W2_O : W2_O + E] = W2.astype(BF16_NP)
    wcommon[:, WGR_O : WGR_O + H] = Wgr.astype(BF16_NP)
    wcommon[:, WGZ_O : WGZ_O + H] = Wgz.astype(BF16_NP)
    wcommon[:, WGN_O : WGN_O + H] = Wgn.astype(BF16_NP)
    wcommon[:, WHR_O : WHR_O + H] = Whr.astype(BF16_NP)
    wcommon[:, WHZ_O : WHZ_O + H] = Whz.astype(BF16_NP)
    wcommon[:, WHN_O : WHN_O + H] = (0.5 * Whn).astype(BF16_NP)


    wout = np.ascontiguousarray(Wout.astype(BF16_NP))

    obsTp = np.zeros((OBSP, A), BF16_NP)
    obsTp[:OBS] = obs.T.astype(BF16_NP)
    belT = np.ascontiguousarray(beliefs.T, f32)
    belTb = belT.astype(BF16_NP)

    # host belief projection: bf16 compute rounding, fp8 storage for attention
    bp = (beliefs.astype(BF16_NP).astype(f32) @ Wb.astype(BF16_NP).astype(f32) + bb)
    bp16 = bp.astype(BF16_NP)                      # [A, E]
    bp8 = bp16.astype(f32).astype(F8_NP)           # fp8 attention operand
    bp8f = bp8.astype(f32)
    bpT8 = np.ascontiguousarray(bp8.T)             # [E, A] fp8
    bpT2 = np.concatenate([bpT8, bpT8], axis=0)    # [128, A] duplicated halves
    # bp_aug: per key tile [128, 80] = [bp rows | ones | pad], fp8; 80-col
    # slots make DoubleRow pair strides 16-aligned
    bpaug = np.zeros((128, JT * JSTR), F8_NP)
    bpaug3 = bpaug.reshape(128, JT, JSTR)
    bpaug3[:, :, 0:E] = bp8.reshape(JT, 128, E).transpose(1, 0, 2)
    bpaug3[:, :, E] = 1.0
    # diag correction terms, reproducing the device's fp8 rounding exactly
    selfdot = (bp8f ** 2).sum(axis=1)
    expd8 = np.exp(0.125 * selfdot).astype(F8_NP).astype(f32)
    expd = expd8[None, :]                          # [1, A] f32
    bpexpT = (bp8f * expd8[:, None]).T.astype(BF16_NP)  # [E, A]

    # per-partition bias columns for ACT / DVE fused bias application
    bcol = np.zeros((128, 8), f32)
    bcol[:, 0] = 0.5 * (bhr + bgr)
    bcol[:, 1] = 0.5 * (bhz + bgz)
    bcol[:, 2] = bgn
    bcol[0:E, 3] = b1
    bcol[0:E, 4] = b2
    bcol[0:NACT, 5] = bout
    bcol[:, 6] = 0.5 * bhn

    in_maps = []
    for c in range(CORES):
        asl = slice(c * AL, (c + 1) * AL)
        wpack = np.concatenate([wcommon, belTb[:, asl]], axis=1)
        obsT_c = np.ascontiguousarray(
            obsTp[:, asl].reshape(KC, 128, AL).transpose(1, 0, 2).reshape(128, KC * AL)
        )
        bpTl = bpT8[:, asl]
        bpTl2 = np.concatenate([bpTl, bpTl], axis=0)  # [128, AL] fp8
        in_maps.append(
            {
                "obsT": obsT_c,
                "belT": np.ascontiguousarray(belT[:, asl]),
                "wpack": np.ascontiguousarray(wpack),
                "wout": wout,
                "bpT2": bpT2,
                "bpaug": bpaug,
                "bpTl2": np.ascontiguousarray(bpTl2),
                "expd": np.ascontiguousarray(expd[:, asl]),
                "bpexp": np.ascontiguousarray(bpexpT[:, asl]),
                "bcol": bcol,
            }
        )
    return in_maps


def run_sharded(inputs, trace=False, **kw):
    """Run the device kernel; returns (logits, new_beliefs, BassKernelResults)."""
    nc = _get_nc()
    in_maps = _prep_inputs(inputs)
    res = run_bass_kernel_spmd(
        nc, in_maps, core_ids=list(range(CORES)), trace=trace, **kw
    )
    logits = np.empty((A, NACT), np.float32)
    new_beliefs = np.empty((A, H), np.float32)
    for c, r in enumerate(res.results):
        asl = slice(c * AL, (c + 1) * AL)
        new_beliefs[asl] = r["out_nbT"].T
        logits[asl] = r["out_lg"].T
    return logits, new_beliefs, res


def kernel(**inputs):
    logits, new_beliefs, _ = run_sharded(inputs)
    return logits, new_beliefs


# revision 21
# speedup vs baseline: 1.0732x; 1.0732x over previous
"""AERIALAgent distributed Trainium2 kernel (8 NeuronCores).

Strategy (row/data parallel over the agent axis):
  - Each core owns 1024 of the 8192 agents: obs MLP, attention rows, GRU
    and outputs for its slice.
  - The belief projection bp = beliefs @ Wb + bb (8192x64, 134 MFLOP ~ 0.7%
    of total work) is computed once on the host in bf16 and REPLICATED to
    all cores in the layouts the attention needs (bpT, bp_aug tiles). In
    this environment a 1MB AllGather costs 60-90us (ncfw rendezvous barrier
    + RDH), dwarfing the projection itself, so replication beats the
    collective by a wide margin and removes all cross-core dependencies.
  - The 8192x8192 attention is flash-style per core with TRANSPOSED score
    tiles ST[j, a] so the softmax-weighted accumulation needs no on-device
    transposes. QK pairs run concurrently on the PE via row tiling
    (tile_position (0,0)/(64,0), K=64 each).
  - Softmax runs without max subtraction (logits in [-3, 6]); the diagonal
    mask is applied algebraically by subtracting exp(|bp_a|^2/8) terms from
    the context numerator and denominator after accumulation.
  - GRU runs fully in transposed [hidden, agent] layout with composite
    weights (Wg @ Wi*) folded on the host; sigmoid is computed via tanh so
    one ACT table set (exp/tanh/relu/copy) serves the whole kernel.
  - Attention matmuls and the exp output run in fp8-e4m3 (f32 PSUM
    accumulation); the PV pair uses the fp8 DoubleRow perf mode. GRU/MLP
    matmuls are bf16. Outputs return transposed; the host transposes back.
"""

import numpy as np
import ml_dtypes

import concourse.bass as bass
import concourse.bacc as bacc
import concourse.mybir as mybir
import concourse.tile as tile
from concourse.bass_utils import run_bass_kernel_spmd

BF16_NP = ml_dtypes.bfloat16
F8_NP = ml_dtypes.float8_e4m3
F32 = mybir.dt.float32
BF16 = mybir.dt.bfloat16
F8 = mybir.dt.float8e4
AF = mybir.ActivationFunctionType
ALU = mybir.AluOpType

CORES = 8
A = 8192           # total agents
AL = A // CORES    # 1024 agents per core
OBS = 520
OBSP = 640         # obs dim padded to 5 * 128
KC = OBSP // 128   # 5 contraction chunks for the obs matmul
E = 64             # embed
H = 128            # hidden
NACT = 6
JT = A // 128      # 64 key tiles
NSB = AL // 512    # 2 superblocks of 512 query agents
JSTR = 80          # bp_aug per-tile column stride (64 data + ones + pad, 16-aligned for DoubleRow)
NBPT = 4           # bpT2 / bp_aug split into 4 tiles for DMA/compute overlap

# wpack column offsets (bf16 [128, WCOLS])
W1_O = 0              # 5 chunks of 64
W2_O = 320
WGR_O = 384
WGZ_O = 512
WGN_O = 640
WHR_O = 768
WHZ_O = 896
WHN_O = 1024          # 0.5 * Whn
BELB_O = 1152         # own beliefs.T bf16 [128, 1024]
WCOLS = BELB_O + AL

# brow column offsets (bf16 [1, BCOLS])
B1_O = 0
B2_O = 64
BR_O = 128            # bhr + bg @ Wir
BZ_O = 256            # bhz + bg @ Wiz
BHN_O = 384           # 0.5 * bhn
BGN_O = 512           # bg @ Win
BOUT_O = 640
BCOLS = 648


def _build_nc():
    nc = bacc.Bacc(
        "TRN2",
        target_bir_lowering=False,
        debug=False,
        num_devices=CORES,
    )

    obsT_d = nc.dram_tensor("obsT", [128, KC * AL], BF16, kind="ExternalInput")
    belT_d = nc.dram_tensor("belT", [H, AL], F32, kind="ExternalInput")
    wpack_d = nc.dram_tensor("wpack", [128, WCOLS], BF16, kind="ExternalInput")
    wout_d = nc.dram_tensor("wout", [H, NACT], BF16, kind="ExternalInput")
    bpT2_d = nc.dram_tensor("bpT2", [128, A], F8, kind="ExternalInput")
    bpaug_d = nc.dram_tensor("bpaug", [128, JT * JSTR], F8, kind="ExternalInput")
    bpTl2_d = nc.dram_tensor("bpTl2", [128, AL], F8, kind="ExternalInput")
    expd_d = nc.dram_tensor("expd", [1, AL], F32, kind="ExternalInput")
    bpexp_d = nc.dram_tensor("bpexp", [E, AL], BF16, kind="ExternalInput")
    bcol_d = nc.dram_tensor("bcol", [128, 8], F32, kind="ExternalInput")

    out_nbT_d = nc.dram_tensor("out_nbT", [H, AL], F32, kind="ExternalOutput")
    out_lg_d = nc.dram_tensor("out_lg", [NACT, AL], F32, kind="ExternalOutput")

    JPT = JT // NBPT  # key tiles per bpT2/bpaug tile

    with tile.TileContext(nc) as tc:
        with (
            tc.tile_pool(name="const", bufs=1) as const,
            tc.tile_pool(name="work", bufs=2) as work,
            tc.tile_pool(name="expp", bufs=4) as expp,
            tc.tile_pool(name="pst", bufs=2, space=bass.MemorySpace.PSUM) as pst,
            tc.tile_pool(name="pctx", bufs=1, space=bass.MemorySpace.PSUM) as pctx,
            tc.tile_pool(name="pmisc", bufs=1, space=bass.MemorySpace.PSUM) as pmisc,
        ):
            # ---------------- inputs (attention-critical DMAs first) ----------------
            bpTl2_sb = const.tile([128, AL], F8, name="bpTl2_sb")
            nc.sync.dma_start(bpTl2_sb[:], bpTl2_d[:])
            bpT2_sb = const.tile([128, A], F8, name="bpT2_sb")
            bpaug_sb = const.tile([128, JT * JSTR], F8, name="bpaug_sb")
            for lo, hi in ((0, 1024), (1024, 4096), (4096, 8192)):
                nc.sync.dma_start(bpT2_sb[:, lo:hi], bpT2_d[:, lo:hi])
                jlo, jhi = lo // 128 * JSTR, hi // 128 * JSTR
                nc.sync.dma_start(bpaug_sb[:, jlo:jhi], bpaug_d[:, jlo:jhi])

            w_sb = const.tile([128, WCOLS], BF16, name="w_sb")
            obsT_sb = const.tile([128, KC * AL], BF16, name="obsT_sb")
            belT_sb = const.tile([H, AL], F32, name="belT_sb")
            wout_sb = const.tile([H, NACT], BF16, name="wout_sb")
            expd_sb = const.tile([1, AL], F32, name="expd_sb")

            nc.sync.dma_start(w_sb[:], wpack_d[:])
            nc.sync.dma_start(obsT_sb[:], obsT_d[:])
            nc.gpsimd.dma_start(belT_sb[:], belT_d[:])
            nc.gpsimd.dma_start(wout_sb[:], wout_d[:])
            nc.gpsimd.dma_start(expd_sb[:], expd_d[:])
            bpexp_sb = const.tile([E, AL], BF16, name="bpexp_sb")
            bcol_sb = const.tile([128, 8], F32, name="bcol_sb")
            nc.gpsimd.dma_start(bpexp_sb[:], bpexp_d[:])
            nc.gpsimd.dma_start(bcol_sb[:], bcol_d[:])

            ones_sb = const.tile([1, 512], BF16, name="ones_sb")
            nc.vector.memset(ones_sb[:], 1.0)

            h1T_sb = const.tile([E, AL], BF16, name="h1T_sb")
            concatT_sb = const.tile([H, AL], BF16, name="concatT_sb")
            nbT_sb = const.tile([H, AL], F32, name="nbT_sb")
            lg_sb = const.tile([NACT, AL], F32, name="lg_sb")
            den_sb = const.tile([1, AL], F32, name="den_sb")
            recip_sb = const.tile([1, AL], F32, name="recip_sb")
            recipb_sb = const.tile([1, AL], BF16, name="recipb_sb")

            ctx_tiles = [
                pctx.tile([128, 512], F32, name=f"ctx{sb}", tag="ctx")
                for sb in range(NSB)
            ]

            NG = (JT + 2) // 3  # attention groups of up to 3 key tiles

            def attn_groups(sb, g_lo, g_hi):
                """Emit attention groups [g_lo, g_hi) for superblock sb.

                QK: K=64 matmuls, pairs overlapped in PE row halves.
                PV: one fp8 DoubleRow pair + one regular matmul."""
                ctx = ctx_tiles[sb]
                cs = slice(sb * 512, (sb + 1) * 512)
                for g in range(g_lo, g_hi):
                    jts = list(range(3 * g, min(3 * g + 3, JT)))
                    st = pst.tile([128, 1536], F32, name="st", tag="st")
                    for i, jt in enumerate(jts):
                        half = i % 2  # alternate PE row groups so pairs overlap
                        nc.tensor.matmul(
                            st[:, i * 512 : (i + 1) * 512],
                            bpT2_sb[
                                half * E : (half + 1) * E,
                                jt * 128 : (jt + 1) * 128,
                            ],
                            bpTl2_sb[half * E : (half + 1) * E, cs],
                            start=True,
                            stop=True,
                            tile_position=(half * E, 0),
                        )
                    ex = expp.tile([128, 1536], F8, name="ex")
                    nc.scalar.activation(
                        ex[:, 0 : len(jts) * 512],
                        st[:, 0 : len(jts) * 512],
                        AF.Exp,
                        scale=0.125,
                    )
                    if len(jts) >= 2:
                        # PV for jts[0], jts[1] fused in one DoubleRow matmul
                        nc.tensor.matmul(
                            ctx[0 : E + 1, :],
                            bpaug_sb[
                                :, jts[0] * JSTR : jts[0] * JSTR + 160
                            ].rearrange("p (i m) -> p i m", i=2)[:, :, 0 : E + 1],
                            ex[:, 0:1024].rearrange("p (i n) -> p i n", i=2),
                            start=(jts[0] == 0),
                            stop=(jts[-1] == JT - 1 and len(jts) == 2),
                            perf_mode=mybir.MatmulPerfMode.DoubleRow,
                        )
                    for i, jt in enumerate(jts[2:], start=2):
                        nc.tensor.matmul(
                            ctx[0 : E + 1, :],
                            bpaug_sb[:, jt * JSTR : jt * JSTR + E + 1],
                            ex[:, i * 512 : (i + 1) * 512],
                            start=(jt == 0),
                            stop=(jt == JT - 1),
                        )
                    if len(jts) == 1:
                        nc.tensor.matmul(
                            ctx[0 : E + 1, :],
                            bpaug_sb[:, jts[0] * JSTR : jts[0] * JSTR + E + 1],
                            ex[:, 0:512],
                            start=(jts[0] == 0),
                            stop=(jts[0] == JT - 1),
                        )

            def denom_fix(sb):
                """Denominator (diag removed) and its reciprocal — DVE only."""
                cs = slice(sb * 512, (sb + 1) * 512)
                ctx = ctx_tiles[sb]
                nc.vector.tensor_tensor(
                    den_sb[:, cs], ctx[E : E + 1, :], expd_sb[:, cs], ALU.subtract
                )
                nc.vector.reciprocal_approx_fast(recip_sb[:, cs], den_sb[:, cs])
                nc.vector.tensor_copy(recipb_sb[:, cs], recip_sb[:, cs])

            def obs_mlp():
                # h2T = relu(W2.T relu(W1.T obsT + b1) + b2) -> concatT rows 0..63
                for ch in range(2):
                    cs = slice(ch * 512, (ch + 1) * 512)
                    p1 = pmisc.tile([128, 512], F32, name="pob1", tag="pm")
                    for kc in range(KC):
                        nc.tensor.matmul(
                            p1[0:E, :],
                            w_sb[:, W1_O + kc * E : W1_O + (kc + 1) * E],
                            obsT_sb[:, kc * AL + ch * 512 : kc * AL + (ch + 1) * 512],
                            start=(kc == 0),
                            stop=(kc == KC - 1),
                        )
                    nc.vector.tensor_scalar(
                        h1T_sb[:, cs], p1[0:E, :], bcol_sb[0:E, 3:4], 0.0,
                        ALU.add, ALU.max,
                    )
                for ch in range(2):
                    cs = slice(ch * 512, (ch + 1) * 512)
                    p2 = pmisc.tile([128, 512], F32, name="pob2", tag="pm")
                    nc.tensor.matmul(
                        p2[0:E, :],
                        w_sb[0:E, W2_O : W2_O + E],
                        h1T_sb[:, cs],
                        start=True,
                        stop=True,
                    )
                    nc.vector.tensor_scalar(
                        concatT_sb[0:E, cs], p2[0:E, :], bcol_sb[0:E, 4:5], 0.0,
                        ALU.add, ALU.max,
                    )

            def tail_fix(sb):
                """Normalize the context and write it into concatT rows 64..127."""
                cs = slice(sb * 512, (sb + 1) * 512)
                ctx = ctx_tiles[sb]
                rb = pmisc.tile([128, 512], F32, name="prb", tag="pm")
                nc.tensor.matmul(
                    rb[0:E, :], ones_sb[:, 0:E], recipb_sb[:, cs], start=True, stop=True
                )
                t_sb = work.tile([E, 512], F32, name="t_sb")
                nc.vector.tensor_tensor(t_sb[:], ctx[0:E, :], bpexp_sb[:, cs], ALU.subtract)
                nc.vector.tensor_tensor(
                    concatT_sb[E:H, cs], t_sb[:], rb[0:E, :], ALU.mult
                )

            def tail_gru(sb):
                """GRU chain producing new beliefs (transposed layout).

                The first superblock's chain overlaps attention and uses the
                single misc PSUM bank; the final one runs after attention, so
                it borrows the (now idle) st pool for pipelining."""
                cs = slice(sb * 512, (sb + 1) * 512)
                belb_rhs = w_sb[:, BELB_O + sb * 512 : BELB_O + (sb + 1) * 512]
                last = sb == NSB - 1

                def gpsum(nm):
                    if last:
                        return pst.tile([128, 512], F32, name=nm + "s", tag="st")
                    return pmisc.tile([128, 512], F32, name=nm, tag="pm")

                # u-gate first: it only needs beliefs, so it runs while the
                # context fix is still finishing
                pu = gpsum("pgu")
                nc.tensor.matmul(pu[:], w_sb[:, WHN_O : WHN_O + H], belb_rhs,
                                 start=True, stop=True)
                u2_sb = work.tile([H, 512], F32, name="u2_sb")
                nc.vector.tensor_scalar(u2_sb[:], pu[:], bcol_sb[:, 6:7], None, ALU.add)

                pr = gpsum("pgr")
                nc.tensor.matmul(pr[:], w_sb[:, WGR_O : WGR_O + H], concatT_sb[:, cs],
                                 start=True, stop=False)
                nc.tensor.matmul(pr[:], w_sb[:, WHR_O : WHR_O + H], belb_rhs,
                                 start=False, stop=True)
                tr_sb = work.tile([H, 512], F32, name="tr_sb")
                nc.scalar.activation(tr_sb[:], pr[:], AF.Tanh, bias=bcol_sb[:, 0:1], scale=0.5)

                pz = gpsum("pgz")
                nc.tensor.matmul(pz[:], w_sb[:, WGZ_O : WGZ_O + H], concatT_sb[:, cs],
                                 start=True, stop=False)
                nc.tensor.matmul(pz[:], w_sb[:, WHZ_O : WHZ_O + H], belb_rhs,
                                 start=False, stop=True)
                tz_sb = work.tile([H, 512], F32, name="tz_sb")
                nc.scalar.activation(tz_sb[:], pz[:], AF.Tanh, bias=bcol_sb[:, 1:2], scale=0.5)

                pn = gpsum("pgn")
                nc.tensor.matmul(pn[:], w_sb[:, WGN_O : WGN_O + H], concatT_sb[:, cs],
                                 start=True, stop=True)
                # v = (tr + 1) * u'   (u' = 0.5*(bel@Whn + bhn))
                v_sb = work.tile([H, 512], F32, name="v_sb")
                nc.vector.scalar_tensor_tensor(
                    v_sb[:], tr_sb[:], 1.0, u2_sb[:], ALU.add, ALU.mult
                )
                t2_sb = work.tile([H, 512], F32, name="t2_sb")
                nc.vector.tensor_tensor(t2_sb[:], v_sb[:], pn[:], ALU.add)
                n_sb = work.tile([H, 512], F32, name="n_sb")
                nc.scalar.activation(n_sb[:], t2_sb[:], AF.Tanh, bias=bcol_sb[:, 2:3])

                # new_beliefs = n + 0.5*(tz + 1)*(beliefs - n)
                d_sb = work.tile([H, 512], F32, name="d_sb")
                nc.vector.tensor_tensor(d_sb[:], belT_sb[:, cs], n_sb[:], ALU.subtract)
                w2_sb = work.tile([H, 512], F32, name="w2_sb")
                nc.vector.scalar_tensor_tensor(
                    w2_sb[:], tz_sb[:], 1.0, d_sb[:], ALU.add, ALU.mult
                )
                nc.vector.scalar_tensor_tensor(
                    nbT_sb[:, cs], w2_sb[:], 0.5, n_sb[:], ALU.mult, ALU.add
                )

            def tail_out(sb):
                cs = slice(sb * 512, (sb + 1) * 512)
                # logits, transposed: lgT = Wout.T @ nbT + bout
                nbTb_sb = work.tile([H, 512], BF16, name="nbTb_sb")
                nc.vector.tensor_copy(nbTb_sb[:], nbT_sb[:, cs])
                plg = pmisc.tile([128, 512], F32, name="plg", tag="pm")
                nc.tensor.matmul(plg[0:NACT, :], wout_sb[:], nbTb_sb[:],
                                 start=True, stop=True)
                nc.scalar.activation(
                    lg_sb[:, cs], plg[0:NACT, :], AF.Identity,
                    bias=bcol_sb[0:NACT, 5:6],
                )
                nc.sync.dma_start(out_nbT_d[:, cs], nbT_sb[:, cs])
                nc.sync.dma_start(out_lg_d[:, cs], lg_sb[:, cs])

            # Emission order: keep the PE queue dense with attention while the
            # sb0 tail's dependencies resolve, then slot tail work into the
            # middle of sb1's stream so DVE/ACT tail ops overlap attention.
            attn_groups(0, 0, NG)
            denom_fix(0)
            tail_fix(0)
            obs_mlp()
            attn_groups(1, 0, 6)
            tail_gru(0)
            attn_groups(1, 6, NG)
            denom_fix(1)
            tail_fix(1)
            tail_out(0)
            tail_gru(1)
            tail_out(1)

    nc.compile()
    return nc


_NC_CACHE = {}


def _get_nc():
    if "nc" not in _NC_CACHE:
        _NC_CACHE["nc"] = _build_nc()
    return _NC_CACHE["nc"]


def _prep_inputs(inputs):
    f32 = np.float32
    obs = np.asarray(inputs["obs"], f32)
    beliefs = np.asarray(inputs["beliefs"], f32)
    W1 = np.asarray(inputs["W1"], f32)
    b1 = np.asarray(inputs["b1"], f32)
    W2 = np.asarray(inputs["W2"], f32)
    b2 = np.asarray(inputs["b2"], f32)
    Wb = np.asarray(inputs["Wb"], f32)
    bb = np.asarray(inputs["bb"], f32)
    Wg = np.asarray(inputs["Wg"], f32)
    bg = np.asarray(inputs["bg"], f32)
    Wir = np.asarray(inputs["Wir"], f32)
    Wiz = np.asarray(inputs["Wiz"], f32)
    Win = np.asarray(inputs["Win"], f32)
    Whr = np.asarray(inputs["Whr"], f32)
    bhr = np.asarray(inputs["bhr"], f32)
    Whz = np.asarray(inputs["Whz"], f32)
    bhz = np.asarray(inputs["bhz"], f32)
    Whn = np.asarray(inputs["Whn"], f32)
    bhn = np.asarray(inputs["bhn"], f32)
    Wout = np.asarray(inputs["Wout"], f32)
    bout = np.asarray(inputs["bout"], f32)

    # composite GRU input weights (gru_in never materializes on device)
    Wgr = Wg @ Wir
    Wgz = Wg @ Wiz
    Wgn = Wg @ Win
    bgr = bg @ Wir
    bgz = bg @ Wiz
    bgn = bg @ Win

    wcommon = np.zeros((128, BELB_O), BF16_NP)
    W1p = np.zeros((OBSP, E), f32)
    W1p[:OBS] = W1
    wcommon[:, W1_O : W1_O + KC * E] = (
        W1p.reshape(KC, 128, E).transpose(1, 0, 2).reshape(128, KC * E).astype(BF16_NP)
    )
    wcommon[0:E, W2_O : W2_O + E] = W2.astype(BF16_NP)
    wcommon[:, WGR_O : WGR_O + H] = Wgr.astype(BF16_NP)
    wcommon[:, WGZ_O : WGZ_O + H] = Wgz.astype(BF16_NP)
    wcommon[:, WGN_O : WGN_O + H] = Wgn.astype(BF16_NP)
    wcommon[:, WHR_O : WHR_O + H] = Whr.astype(BF16_NP)
    wcommon[:, WHZ_O : WHZ_O + H] = Whz.astype(BF16_NP)
    wcommon[:, WHN_O : WHN_O + H] = (0.5 * Whn).astype(BF16_NP)


    wout = np.ascontiguousarray(Wout.astype(BF16_NP))

    obsTp = np.zeros((OBSP, A), BF16_NP)
    obsTp[:OBS] = obs.T.astype(BF16_NP)
    belT = np.ascontiguousarray(beliefs.T, f32)
    belTb = belT.astype(BF16_NP)

    # host belief projection: bf16 compute rounding, fp8 storage for attention
    bp = (beliefs.astype(BF16_NP).astype(f32) @ Wb.astype(BF16_NP).astype(f32) + bb)
    bp16 = bp.astype(BF16_NP)                      # [A, E]
    bp8 = bp16.astype(f32).astype(F8_NP)           # fp8 attention operand
    bp8f = bp8.astype(f32)
    bpT8 = np.ascontiguousarray(bp8.T)             # [E, A] fp8
    bpT2 = np.concatenate([bpT8, bpT8], axis=0)    # [128, A] duplicated halves
    # bp_aug: per key tile [128, 80] = [bp rows | ones | pad], fp8; 80-col
    # slots make DoubleRow pair strides 16-aligned
    bpaug = np.zeros((128, JT * JSTR), F8_NP)
    bpaug3 = bpaug.reshape(128, JT, JSTR)
    bpaug3[:, :, 0:E] = bp8.reshape(JT, 128, E).transpose(1, 0, 2)
    bpaug3[:, :, E] = 1.0
    # diag correction terms, reproducing the device's fp8 rounding exactly
    selfdot = (bp8f ** 2).sum(axis=1)
    expd8 = np.exp(0.125 * selfdot).astype(F8_NP).astype(f32)
    expd = expd8[None, :]                          # [1, A] f32
    bpexpT = (bp8f * expd8[:, None]).T.astype(BF16_NP)  # [E, A]

    # per-partition bias columns for ACT / DVE fused bias application
    bcol = np.zeros((128, 8), f32)
    bcol[:, 0] = 0.5 * (bhr + bgr)
    bcol[:, 1] = 0.5 * (bhz + bgz)
    bcol[:, 2] = bgn
    bcol[0:E, 3] = b1
    bcol[0:E, 4] = b2
    bcol[0:NACT, 5] = bout
    bcol[:, 6] = 0.5 * bhn

    in_maps = []
    for c in range(CORES):
        asl = slice(c * AL, (c + 1) * AL)
        wpack = np.concatenate([wcommon, belTb[:, asl]], axis=1)
        obsT_c = np.ascontiguousarray(
            obsTp[:, asl].reshape(KC, 128, AL).transpose(1, 0, 2).reshape(128, KC * AL)
        )
        bpTl = bpT8[:, asl]
        bpTl2 = np.concatenate([bpTl, bpTl], axis=0)  # [128, AL] fp8
        in_maps.append(
            {
                "obsT": obsT_c,
                "belT": np.ascontiguousarray(belT[:, asl]),
                "wpack": np.ascontiguousarray(wpack),
                "wout": wout,
                "bpT2": bpT2,
                "bpaug": bpaug,
                "bpTl2": np.ascontiguousarray(bpTl2),
                "expd": np.ascontiguousarray(expd[:, asl]),
                "bpexp": np.ascontiguousarray(bpexpT[:, asl]),
                "bcol": bcol,
            }
        )
    return in_maps


def run_sharded(inputs, trace=False, **kw):
    """Run the device kernel; returns (logits, new_beliefs, BassKernelResults)."""
    nc = _get_nc()
    in_maps = _prep_inputs(inputs)
    res = run_bass_kernel_spmd(
        nc, in_maps, core_ids=list(range(CORES)), trace=trace, **kw
    )
    logits = np.empty((A, NACT), np.float32)
    new_beliefs = np.empty((A, H), np.float32)
    for c, r in enumerate(res.results):
        asl = slice(c * AL, (c + 1) * AL)
        new_beliefs[asl] = r["out_nbT"].T
        logits[asl] = r["out_lg"].T
    return logits, new_beliefs, res


def kernel(**inputs):
    logits, new_beliefs, _ = run_sharded(inputs)
    return logits, new_beliefs
